# revision 3
# baseline (speedup 1.0000x reference)
# Triplet-margin loss kernel for Trainium2 (Bass/Tile), batch-sharded
# across 8 NeuronCores.
#
# reference math (torch F.pairwise_distance semantics):
#   d_ap[b,p] = || anc[b] - pos[b,p] + eps ||_2
#   d_an[b,n] = || anc[b] - neg[b,n] + eps ||_2
#   loss = mean_{b,p,n} max(d_ap[b,p] - d_an[b,n] + margin, 0)
#
# With a' = anc + eps, each of the 24 distance columns ("slices") per
# 128-row batch tile takes one of three engine paths, chosen to balance
# busy time across Vector/Scalar/GpSimd under the DMA roofline:
#   T1: dot = a'.x on DVE (STT w/ fp32 accum), nrm = ||x||^2 on ACT
#       (activation Square w/ accum); d = sqrt((nrm - 2 dot) + ||a'||^2)
#   Pool slices: u = x - a' on GpSimd (one op per 4-slice chunk with a
#       broadcast a'), then d^2 = sum u^2 reduced on DVE (T4 slices,
#       STT mult w/ accum) or ACT (T3 slices, Square w/ accum); d =
#       sqrt(d^2) with no bias.
# The (p,n) pair combination is two ops on [128, P*N]: a broadcast
# subtract (s_p - d_an_n) on DVE and a Relu w/ accum on ACT, giving the
# per-partition loss sum per tile. Each core returns [128, NT] partial
# sums; the host sums and scales.

import numpy as np

import concourse.bacc as bacc
import concourse.mybir as mybir
import concourse.tile as tile
from concourse import bass_utils

B, Z = 2048, 1024
NUM_POS, NUM_NEG = 8, 16
NJ = NUM_POS + NUM_NEG
MARGIN, EPS = 1.0, 1e-6
N_CORES = 8
BL = B // N_CORES  # 256 rows of anc per core
P = 128
NT = BL // P  # 2 batch-tiles per core
CH = 4  # z-slices per DMA chunk
CHW = CH * Z
NCHUNK = NJ // CH  # 6 chunks per tile

# slice-type split per tile:
#   slices [0, T1_END)        -> T1 (dot on DVE + norm on ACT)
#   slices [T1_END, T4_END)   -> T4 (Pool subtract + sum u^2 on DVE)
#   slices [T4_END, NJ)       -> T3 (Pool subtract + sum u^2 on ACT)
T1_END = 12
T4_END = 18
POOL_CH0 = T1_END // CH  # first chunk owned by Pool (3)

F32 = mybir.dt.float32
AF = mybir.ActivationFunctionType
OP = mybir.AluOpType


def _emit(tc, nc, anc, pos, neg, out):
    v = nc.vector
    act = nc.scalar
    gp = nc.gpsimd
    te = nc.tensor
    pos2 = pos.rearrange("(b j) z -> b (j z)", j=NUM_POS)  # [BL, 8*Z]
    neg2 = neg.rearrange("(b j) z -> b (j z)", j=NUM_NEG)  # [BL, 16*Z]
    # Pool-owned chunks are DMA'd first: GpSimd has the slowest per-slice
    # path, so its data must land early; T1 chunks (DVE+ACT) drain after.
    CHUNK_ORDER = list(range(POOL_CH0, NCHUNK)) + list(range(POOL_CH0))

    def chunk_src(c, b0):
        if c < NUM_POS // CH:
            return pos2[b0 : b0 + P, c * CHW : (c + 1) * CHW]
        cn = c - NUM_POS // CH
        return neg2[b0 : b0 + P, cn * CHW : (cn + 1) * CHW]

    with (
        tc.tile_pool(name="xp", bufs=7) as xp,
        tc.tile_pool(name="up", bufs=3) as up,
        tc.tile_pool(name="apool", bufs=2) as apool,
        tc.tile_pool(name="scp", bufs=1) as scp,
        tc.tile_pool(name="smp", bufs=2) as smp,
        tc.tile_pool(name="opool", bufs=1) as opool,
    ):
        osb = opool.tile([P, NT], F32, name="osb")
        dve_scr = scp.tile([P, Z], F32, name="dve_scr")
        act_scr = scp.tile([P, Z], F32, name="act_scr")
        pair = scp.tile([P, NUM_POS * NUM_NEG], F32, name="pair")
        pair_scr = scp.tile([P, NUM_POS * NUM_NEG], F32, name="pair_scr")
        eps_t = opool.tile([P, 1], F32, name="eps_t")
        v.memset(eps_t[:, :], EPS)
        for t in range(NT):
            b0 = t * P
            anc_in = apool.tile([P, Z], F32, name="anc_in")
            aprime = apool.tile([P, Z], F32, name="aprime")
            a_nrm = smp.tile([P, 1], F32, name="a_nrm")
            dot = smp.tile([P, T1_END], F32, name="dot")
            nrm = smp.tile([P, NJ], F32, name="nrm")
            d2c = smp.tile([P, T1_END], F32, name="d2c")
            dt_ = smp.tile([P, NJ], F32, name="dt_")
            s_m = smp.tile([P, NUM_POS], F32, name="s_m")

            nc.sync.dma_start(anc_in[:, :], anc[b0 : b0 + P, :])
            act.activation(
                aprime[:, :], anc_in[:, :], AF.Identity, bias=eps_t[:, 0:1], scale=1.0
            )
            act.activation(
                act_scr[:, :], aprime[:, :], AF.Square, accum_out=a_nrm[:, 0:1]
            )

            chunks = {}
            for c in CHUNK_ORDER:
                xt = xp.tile([P, CHW], F32, name="xt")
                nc.sync.dma_start(xt[:, :], chunk_src(c, b0))
                chunks[c] = xt

            # GpSimd: u = x - a' for all Pool-owned chunks, one op per chunk
            ap_b = aprime[:, None, :].broadcast_to([P, CH, Z])
            uts = {}
            for c in range(POOL_CH0, NCHUNK):
                ut = up.tile([P, CHW], F32, name="ut")
                gp.tensor_tensor(
                    out=ut[:, :].rearrange("p (c z) -> p c z", c=CH),
                    in0=chunks[c][:, :].rearrange("p (c z) -> p c z", c=CH),
                    in1=ap_b,
                    op=OP.subtract,
                )
                uts[c] = ut

            # DVE reductions: T4 slices (sum u^2), then T1 dots in DMA order
            for jj in range(T1_END, T4_END):
                us = uts[jj // CH][:, (jj % CH) * Z : (jj % CH + 1) * Z]
                v.scalar_tensor_tensor(
                    out=dve_scr[:, :],
                    in0=us,
                    scalar=1.0,
                    in1=us,
                    op0=OP.bypass,
                    op1=OP.mult,
                    accum_out=nrm[:, jj : jj + 1],
                )
            for jj in range(T1_END):
                xs = chunks[jj // CH][:, (jj % CH) * Z : (jj % CH + 1) * Z]
                v.scalar_tensor_tensor(
                    out=dve_scr[:, :],
                    in0=xs,
                    scalar=1.0,
                    in1=aprime[:, :],
                    op0=OP.bypass,
                    op1=OP.mult,
                    accum_out=dot[:, jj : jj + 1],
                )

            # ACT reductions: T3 slices (sum u^2), then T1 norms in DMA order
            for jj in range(T4_END, NJ):
                us = uts[jj // CH][:, (jj % CH) * Z : (jj % CH + 1) * Z]
                act.activation(
                    act_scr[:, :], us, AF.Square, accum_out=nrm[:, jj : jj + 1]
                )
            for jj in range(T1_END):
                xs = chunks[jj // CH][:, (jj % CH) * Z : (jj % CH + 1) * Z]
                act.activation(
                    act_scr[:, :], xs, AF.Square, accum_out=nrm[:, jj : jj + 1]
                )

            # d^2 for T1 cols: nrm - 2*dot, then d = sqrt(d^2 + ||a'||^2)
            v.scalar_tensor_tensor(
                out=d2c[:, :],
                in0=dot[:, :],
                scalar=-2.0,
                in1=nrm[:, 0:T1_END],
                op0=OP.mult,
                op1=OP.add,
            )
            act.activation(
                dt_[:, 0:T1_END], d2c[:, :], AF.Sqrt, bias=a_nrm[:, 0:1], scale=1.0
            )
            # Pool cols already hold d^2 in nrm
            act.activation(dt_[:, T1_END:NJ], nrm[:, T1_END:NJ], AF.Sqrt)
            # s = d_ap + margin
            v.tensor_scalar_add(s_m[:, :], dt_[:, 0:NUM_POS], MARGIN)
            # pair[p,n] = s_p - d_an_n ; loss sum = sum relu(pair)
            v.scalar_tensor_tensor(
                out=pair[:, :].rearrange("p (a b) -> p a b", a=NUM_POS),
                in0=s_m[:, :, None].broadcast_to([P, NUM_POS, NUM_NEG]),
                scalar=1.0,
                in1=dt_[:, None, NUM_POS:NJ].broadcast_to([P, NUM_POS, NUM_NEG]),
                op0=OP.bypass,
                op1=OP.subtract,
            )
            act.activation(
                pair_scr[:, :], pair[:, :], AF.Relu, accum_out=osb[:, t : t + 1]
            )
        nc.sync.dma_start(out[:, :], osb[:, :])


_NC_CACHE = None


def build():
    global _NC_CACHE
    if _NC_CACHE is None:
        nc = bacc.Bacc(
            "TRN2", target_bir_lowering=False, debug=False, num_devices=N_CORES
        )
        anc = nc.dram_tensor("anc", (BL, Z), F32, kind="ExternalInput").ap()
        pos = nc.dram_tensor("pos", (BL * NUM_POS, Z), F32, kind="ExternalInput").ap()
        neg = nc.dram_tensor("neg", (BL * NUM_NEG, Z), F32, kind="ExternalInput").ap()
        out = nc.dram_tensor("out", (P, NT), F32, kind="ExternalOutput").ap()
        with tile.TileContext(nc) as tc:
            _emit(tc, nc, anc, pos, neg, out)
        nc.compile()
        _NC_CACHE = nc
    return _NC_CACHE


def make_in_maps(anc_embedding, pos_embedding, neg_embedding):
    anc_embedding = np.asarray(anc_embedding, dtype=np.float32)
    pos_embedding = np.asarray(pos_embedding, dtype=np.float32)
    neg_embedding = np.asarray(neg_embedding, dtype=np.float32)
    in_maps = []
    for c in range(N_CORES):
        in_maps.append(
            {
                "anc": np.ascontiguousarray(anc_embedding[c * BL : (c + 1) * BL]),
                "pos": np.ascontiguousarray(
                    pos_embedding[c * BL * NUM_POS : (c + 1) * BL * NUM_POS]
                ),
                "neg": np.ascontiguousarray(
                    neg_embedding[c * BL * NUM_NEG : (c + 1) * BL * NUM_NEG]
                ),
            }
        )
    return in_maps


def combine(outs):
    # outs: list of [P, NT] per-core partial sums of relu(s - d_an)
    total = sum(o.astype(np.float64).sum() for o in outs)
    return np.float32(total / (B * NUM_POS * NUM_NEG))


def kernel(anc_embedding, pos_embedding, neg_embedding):
    nc = build()
    in_maps = make_in_maps(anc_embedding, pos_embedding, neg_embedding)
    res = bass_utils.run_bass_kernel_spmd(nc, in_maps, core_ids=list(range(N_CORES)))
    return combine([r["out"] for r in res.results])


# revision 4
# speedup vs baseline: 1.3030x; 1.3030x over previous
# Triplet-margin loss kernel for Trainium2 (Bass/Tile), batch-sharded
# across 8 NeuronCores.
#
# reference math (torch F.pairwise_distance semantics):
#   d_ap[b,p] = || anc[b] - pos[b,p] + eps ||_2
#   d_an[b,n] = || anc[b] - neg[b,n] + eps ||_2
#   loss = mean_{b,p,n} max(d_ap[b,p] - d_an[b,n] + margin, 0)
#
# (eps = 1e-6 shifts d^2 by ~3e-8 relative; it is dropped here, far
# below the fp32 noise floor of the d^2 cancellation itself.)
#
# Pure DVE+ACT pipeline (GpSimd streaming shares an SBUF port with DVE
# and slows it ~8x when both run, so it gets no slice work):
#   every slice j: dotm2[b,j] = -2 sum_z x*a  via one DVE STT (scalar
#   -2.0 folded into op0) and nrm[b,j] = sum_z x^2 via one ACT Square
#   w/ accum, except K_DVE slices per tile whose norm runs on DVE
#   (self-multiply STT) to balance the two engines.
#   d = sqrt((nrm + dotm2) + ||a||^2)  -- one TT add + one biased Sqrt.
# The (p,n) pair combination is two ops on [128, P*N]: a broadcast
# (d_ap + margin) - d_an STT on DVE and a Relu w/ accum on ACT, giving
# the per-partition loss sum per tile. Each core returns [128, NT]
# partial sums; the host sums and scales.

import numpy as np

import concourse.bacc as bacc
import concourse.mybir as mybir
import concourse.tile as tile
from concourse import bass_utils

B, Z = 2048, 1024
NUM_POS, NUM_NEG = 8, 16
NJ = NUM_POS + NUM_NEG
MARGIN = 1.0
N_CORES = 8
BL = B // N_CORES  # 256 rows of anc per core
P = 128
NT = BL // P  # 2 batch-tiles per core

# chunk layout per tile: slice counts per DMA; small final chunks keep
# the post-DMA tail short.
CHUNK_SLICES = [4, 4, 4, 4, 4, 2, 2]
# slices whose norm is reduced on DVE instead of ACT (engine balance)
DVE_NRM = {5, 13, 21}

F32 = mybir.dt.float32
AF = mybir.ActivationFunctionType
OP = mybir.AluOpType


def _emit(tc, nc, anc, pos, neg, out):
    v = nc.vector
    act = nc.scalar
    pos2 = pos.rearrange("(b j) z -> b (j z)", j=NUM_POS)  # [BL, 8*Z]
    neg2 = neg.rearrange("(b j) z -> b (j z)", j=NUM_NEG)  # [BL, 16*Z]

    starts = np.cumsum([0] + CHUNK_SLICES).tolist()
    assert starts[-1] == NJ

    def chunk_src(ci, b0):
        j0, j1 = starts[ci], starts[ci + 1]
        if j1 <= NUM_POS:
            return pos2[b0 : b0 + P, j0 * Z : j1 * Z]
        assert j0 >= NUM_POS
        return neg2[b0 : b0 + P, (j0 - NUM_POS) * Z : (j1 - NUM_POS) * Z]

    with (
        tc.tile_pool(name="xp", bufs=9) as xp,
        tc.tile_pool(name="apool", bufs=2) as apool,
        tc.tile_pool(name="scp", bufs=1) as scp,
        tc.tile_pool(name="smp", bufs=2) as smp,
        tc.tile_pool(name="opool", bufs=1) as opool,
    ):
        osb = opool.tile([P, NT], F32, name="osb")
        dve_scr = scp.tile([P, Z], F32, name="dve_scr")
        act_scr = scp.tile([P, Z], F32, name="act_scr")
        pair = scp.tile([P, NUM_POS * NUM_NEG], F32, name="pair")
        pair_scr = scp.tile([P, NUM_POS * NUM_NEG], F32, name="pair_scr")
        for t in range(NT):
            b0 = t * P
            anc_in = apool.tile([P, Z], F32, name="anc_in")
            a_nrm = smp.tile([P, 1], F32, name="a_nrm")
            dotm2 = smp.tile([P, NJ], F32, name="dotm2")
            nrm = smp.tile([P, NJ], F32, name="nrm")
            d2c = smp.tile([P, NJ], F32, name="d2c")
            dt_ = smp.tile([P, NJ], F32, name="dt_")

            nc.sync.dma_start(anc_in[:, :], anc[b0 : b0 + P, :])
            act.activation(
                act_scr[:, :], anc_in[:, :], AF.Square, accum_out=a_nrm[:, 0:1]
            )

            chunks = []
            for ci in range(len(CHUNK_SLICES)):
                xt = xp.tile([P, CHUNK_SLICES[ci] * Z], F32, name="xt")
                nc.sync.dma_start(xt[:, :], chunk_src(ci, b0))
                chunks.append(xt)

            def xs_of(jj):
                ci = next(i for i in range(len(starts) - 1) if starts[i + 1] > jj)
                q = jj - starts[ci]
                return chunks[ci][:, q * Z : (q + 1) * Z]

            # DVE: dotm2[:,jj] = sum((x * -2) * a); DVE-owned norms inline
            for jj in range(NJ):
                xs = xs_of(jj)
                v.scalar_tensor_tensor(
                    out=dve_scr[:, :],
                    in0=xs,
                    scalar=-2.0,
                    in1=anc_in[:, :],
                    op0=OP.mult,
                    op1=OP.mult,
                    accum_out=dotm2[:, jj : jj + 1],
                )
                if jj in DVE_NRM:
                    v.scalar_tensor_tensor(
                        out=dve_scr[:, :],
                        in0=xs,
                        scalar=1.0,
                        in1=xs,
                        op0=OP.bypass,
                        op1=OP.mult,
                        accum_out=nrm[:, jj : jj + 1],
                    )

            # ACT: nrm[:,jj] = sum x^2 for the rest
            for jj in range(NJ):
                if jj in DVE_NRM:
                    continue
                act.activation(
                    act_scr[:, :], xs_of(jj), AF.Square, accum_out=nrm[:, jj : jj + 1]
                )

            # d = sqrt((nrm + dotm2) + ||a||^2)
            v.tensor_tensor(out=d2c[:, :], in0=dotm2[:, :], in1=nrm[:, :], op=OP.add)
            act.activation(
                dt_[:, :], d2c[:, :], AF.Sqrt, bias=a_nrm[:, 0:1], scale=1.0
            )
            # pair[p,n] = (d_ap_p + margin) - d_an_n ; loss sum = sum relu
            v.scalar_tensor_tensor(
                out=pair[:, :].rearrange("p (a b) -> p a b", a=NUM_POS),
                in0=dt_[:, 0:NUM_POS, None].broadcast_to([P, NUM_POS, NUM_NEG]),
                scalar=MARGIN,
                in1=dt_[:, None, NUM_POS:NJ].broadcast_to([P, NUM_POS, NUM_NEG]),
                op0=OP.add,
                op1=OP.subtract,
            )
            act.activation(
                pair_scr[:, :], pair[:, :], AF.Relu, accum_out=osb[:, t : t + 1]
            )
        nc.sync.dma_start(out[:, :], osb[:, :])


_NC_CACHE = None


def build():
    global _NC_CACHE
    if _NC_CACHE is None:
        nc = bacc.Bacc(
            "TRN2", target_bir_lowering=False, debug=False, num_devices=N_CORES
        )
        anc = nc.dram_tensor("anc", (BL, Z), F32, kind="ExternalInput").ap()
        pos = nc.dram_tensor("pos", (BL * NUM_POS, Z), F32, kind="ExternalInput").ap()
        neg = nc.dram_tensor("neg", (BL * NUM_NEG, Z), F32, kind="ExternalInput").ap()
        out = nc.dram_tensor("out", (P, NT), F32, kind="ExternalOutput").ap()
        with tile.TileContext(nc) as tc:
            _emit(tc, nc, anc, pos, neg, out)
        nc.compile()
        _NC_CACHE = nc
    return _NC_CACHE


def make_in_maps(anc_embedding, pos_embedding, neg_embedding):
    anc_embedding = np.asarray(anc_embedding, dtype=np.float32)
    pos_embedding = np.asarray(pos_embedding, dtype=np.float32)
    neg_embedding = np.asarray(neg_embedding, dtype=np.float32)
    in_maps = []
    for c in range(N_CORES):
        in_maps.append(
            {
                "anc": np.ascontiguousarray(anc_embedding[c * BL : (c + 1) * BL]),
                "pos": np.ascontiguousarray(
                    pos_embedding[c * BL * NUM_POS : (c + 1) * BL * NUM_POS]
                ),
                "neg": np.ascontiguousarray(
                    neg_embedding[c * BL * NUM_NEG : (c + 1) * BL * NUM_NEG]
                ),
            }
        )
    return in_maps


def combine(outs):
    # outs: list of [P, NT] per-core partial sums of relu((d_ap+m) - d_an)
    total = sum(o.astype(np.float64).sum() for o in outs)
    return np.float32(total / (B * NUM_POS * NUM_NEG))


def kernel(anc_embedding, pos_embedding, neg_embedding):
    nc = build()
    in_maps = make_in_maps(anc_embedding, pos_embedding, neg_embedding)
    res = bass_utils.run_bass_kernel_spmd(nc, in_maps, core_ids=list(range(N_CORES)))
    return combine([r["out"] for r in res.results])
